# revision 5
# baseline (speedup 1.0000x reference)
"""Trainium2 Bass kernel for nn_MnistModel (MoE transformer, 8 blocks).

Sharding: pure data-parallel over batch (B=8 -> 1 image per NeuronCore).
The residual trunk runs in strict fp32 on-device (routing decisions are
numerically razor-thin: any reduced-precision matmul flips top-2 expert
choices and corrupts the output, so fp32 it is). One compiled block
program (dense MoE: every expert on every token, combined with the
renormalized top-2 gates) is launched 8 times with per-parity weights
kept resident on the devices between launches.
"""
import numpy as np

HID = 1024
E = 8
NH = 16
HD = 64
NB = 8
B = 8
IMG = 28
N = IMG * IMG          # 784 real tokens
NPAD = 896             # padded to 7 x 128
TT = 7                 # token tiles of 128
KT = 8                 # hidden tiles of 128
VOCAB = 256
F = 4096               # ff hidden
FD = 32                # ff hidden tiles
EPS = 1e-5

_cache = {}


# ---------------------------------------------------------------- host prep

def _rope_tables():
    import math
    pos = np.arange(NPAD, dtype=np.float64)[:, None]
    div = np.exp(np.arange(HD // 2, dtype=np.float64) * (-math.log(10000.0) / (HD // 2)))
    emb = pos * div                                        # [NPAD, 32]
    sin = np.concatenate([np.sin(emb), np.sin(emb)], -1)   # [NPAD, 64]
    cos = np.concatenate([np.cos(emb), np.cos(emb)], -1)
    # layout [128 partitions = 2 heads x 64 dims, NPAD]
    c1 = (1.0 + cos).T.astype(np.float32)                  # [64, NPAD]
    s1 = sin.T.astype(np.float32).copy()
    s1[:HD // 2] *= -1.0                                   # sign of rot folded in
    ropeC = np.concatenate([c1, c1], 0).astype(np.float32)     # [128, NPAD]
    ropeS = np.concatenate([s1, s1], 0).astype(np.float32)
    return ropeC, ropeS


def _consts():
    ident = np.eye(128, dtype=np.float32)
    diag = np.where(np.arange(128)[:, None] <= np.arange(128)[None, :],
                    0.0, -1e30).astype(np.float32)  # S^T[kp,qp]: kp<=qp allowed
    iota8 = np.tile(np.arange(8, dtype=np.float32)[None, :], (128, 1))
    ropeC, ropeS = _rope_tables()
    return ident, diag, iota8, ropeC, ropeS


def _np(a):
    return np.asarray(a, dtype=np.float32)


def _fold_ln(gamma, beta, mats, biases):
    """ln out z*g+b feeds each mats[i] (x @ M + biases[i]).
    Return scaled mats and adjusted bias vectors so device can apply raw z."""
    g = _np(gamma)
    b = _np(beta)
    out_m, out_b = [], []
    for M, bias in zip(mats, biases):
        M = _np(M)
        Mg = M if np.all(g == 1.0) else (g[:, None] * M).astype(np.float32)
        extra = None if np.all(b == 0.0) else (b @ M).astype(np.float32)
        bb = _np(bias) if bias is not None else None
        if extra is not None:
            bb = extra if bb is None else (bb + extra).astype(np.float32)
        out_m.append(np.ascontiguousarray(Mg))
        out_b.append(bb)
    return out_m, out_b


def _prep_weights(params):
    """Per-parity flat weight dict for the block program."""
    out = []
    for par in range(2):
        bp = params["blocks"][par]
        w = {}
        # ln1 feeds qw, kw, v-router, v-experts
        vmoe, pmoe, ff = bp["vmoe"], bp["pmoe"], bp["ff"]
        mats = [bp["qw"], bp["kw"], vmoe["rw"]] + [vmoe["w"][e] for e in range(E)]
        biases = [bp["qb"], bp["kb"], vmoe["rb"]] + [vmoe["b"][e] for e in range(E)]
        (m_f, b_f) = _fold_ln(bp["g1"], bp["be1"], mats, biases)
        w["qw"], w["kw"], w["vrw"] = m_f[0], m_f[1], m_f[2]
        w["vw"] = np.stack(m_f[3:11])
        qb = b_f[0] if b_f[0] is not None else np.zeros(HID, np.float32)
        kb = b_f[1] if b_f[1] is not None else np.zeros(HID, np.float32)
        w["qbT"] = np.ascontiguousarray(qb.reshape(KT, 128).T)   # [128, 8]
        w["kbT"] = np.ascontiguousarray(kb.reshape(KT, 128).T)
        w["vrb"] = b_f[2] if b_f[2] is not None else np.zeros(E, np.float32)
        for e in range(E):
            assert b_f[3 + e] is None or np.all(b_f[3 + e] == 0), "nonzero expert bias unsupported"
        # pmoe input = attention out (no ln)
        w["prw"] = _np(pmoe["rw"])
        w["prb"] = _np(pmoe["rb"])
        w["pw"] = _np(pmoe["w"])
        assert np.all(_np(pmoe["b"]) == 0)
        # ln2 feeds ff router + w1
        mats = [ff["rw"]] + [ff["w1"][e] for e in range(E)]
        biases = [ff["rb"]] + [ff["b1"][e] for e in range(E)]
        (m_f, b_f) = _fold_ln(bp["g2"], bp["be2"], mats, biases)
        w["frw"] = m_f[0]
        w["frb"] = b_f[0] if b_f[0] is not None else np.zeros(E, np.float32)
        for e in range(E):
            assert b_f[1 + e] is None or np.all(b_f[1 + e] == 0)
        # fw1 rearranged [E, FD, KT, 128, 128]: tile (d, kt) = w1[kt*128:.., d*128:..]
        w1 = np.stack(m_f[1:])                                   # [E, 1024, 4096]
        w["fw1r"] = np.ascontiguousarray(
            w1.reshape(E, KT, 128, FD, 128).transpose(0, 3, 1, 2, 4))
        w["fw2"] = np.ascontiguousarray(_np(ff["w2"]) * 0.5)     # 0.5 of gelu folded
        assert np.all(_np(ff["b2"]) == 0)
        out.append(w)
    return out


def _embed(x, condition, params):
    iw, ib = params["init"]
    sw, sb = params["start"]
    iw, ib, sw, sb = _np(iw), _np(ib), _np(sw), _np(sb)
    x = _np(x)
    condition = _np(condition)
    b = x.shape[0]
    xs = x.transpose(0, 2, 3, 1).reshape(b, N, 1) @ iw + ib
    cond = condition[:, None, None] @ sw + sb
    xs = np.concatenate([cond, xs[:, :-1, :]], axis=1)       # [B, 784, HID]
    pad = np.zeros((b, NPAD - N, HID), np.float32)
    return np.concatenate([xs, pad], axis=1).astype(np.float32)


# ---------------------------------------------------------------- device program

def _build_block():
    from concourse import bacc, mybir
    from concourse import tile

    F32 = mybir.dt.float32
    U32 = mybir.dt.uint32
    AF = mybir.ActivationFunctionType
    ALU = mybir.AluOpType

    nc = bacc.Bacc()
    P = lambda n, s: nc.declare_dram_parameter(n, s, F32, isOutput=False)
    xs_in = P("xs_in", [NPAD, HID])
    qw = P("qw", [HID, HID]); kw = P("kw", [HID, HID])
    qbT = P("qbT", [128, KT]); kbT = P("kbT", [128, KT])
    vrw = P("vrw", [HID, E]); vrb = P("vrb", [E])
    prw = P("prw", [HID, E]); prb = P("prb", [E])
    frw = P("frw", [HID, E]); frb = P("frb", [E])
    vw = P("vw", [E, HID, HID]); pw = P("pw", [E, HID, HID])
    fw1r = P("fw1r", [E, FD, KT, 128, 128]); fw2 = P("fw2", [E, F, HID])
    ow = P("ow", [HID, VOCAB])
    identc = P("identc", [128, 128]); diagc = P("diagc", [128, 128])
    iota8c = P("iota8c", [128, 8])
    ropeCc = P("ropeCc", [128, NPAD]); ropeSc = P("ropeSc", [128, NPAD])
    xs_out = nc.declare_dram_parameter("xs_out", [NPAD, HID], F32, isOutput=True)
    lg_out = nc.declare_dram_parameter("lg_out", [NPAD, VOCAB], F32, isOutput=True)

    NCH = [(0, 512), (512, 384)]          # token chunks for ff (sum = NPAD)

    with tile.TileContext(nc) as tc:
        with tc.tile_pool(name="const", bufs=1) as cpool, \
             tc.tile_pool(name="main", bufs=1) as main:
            ident = cpool.tile_from(identc[:], name="ident")
            diag = cpool.tile_from(diagc[:], name="diag")
            iota8 = cpool.tile_from(iota8c[:], name="iota8")
            ropeC = cpool.tile_from(ropeCc[:], name="ropeC")
            ropeS = cpool.tile_from(ropeSc[:], name="ropeS")

            xs_sb = main.tile([128, TT, HID], F32, name="xs_sb")
            for t in range(TT):
                nc.sync.dma_start(out=xs_sb[:, t, :], in_=xs_in[t * 128:(t + 1) * 128, :])

            def xT_tile(name):
                return main.tile([128, KT, NPAD], F32, name=name, tag="xT", bufs=1)

            def tok_tile(name):
                return main.tile([128, TT, HID], F32, name=name, tag="tokbuf", bufs=1)

            # ============ helpers ============
            def layernorm(src_sb, dst_sb, scratch):
                """src/dst [128, TT, HID] A-layout."""
                sums = scratch.tile([128, TT], F32, name="ln_sums", tag="ln_sums")
                sqs = scratch.tile([128, TT], F32, name="ln_sqs", tag="ln_sqs")
                sq_scr = scratch.tile([128, HID], F32, name="ln_scr", tag="ln_scr", bufs=2)
                for t in range(TT):
                    nc.vector.tensor_reduce(sums[:, t:t + 1], src_sb[:, t, :],
                                            axis=mybir.AxisListType.X, op=ALU.add)
                    nc.scalar.activation(sq_scr[:], src_sb[:, t, :], AF.Square,
                                         accum_out=sqs[:, t:t + 1])
                    sq_scr = scratch.tile([128, HID], F32, name="ln_scr", tag="ln_scr", bufs=2)
                mu = scratch.tile([128, TT], F32, name="ln_mu", tag="ln_mu")
                var = scratch.tile([128, TT], F32, name="ln_var", tag="ln_var")
                rstd = scratch.tile([128, TT], F32, name="ln_rstd", tag="ln_rstd")
                nc.vector.tensor_scalar_mul(mu[:], sums[:], 1.0 / HID)
                nc.vector.tensor_scalar_mul(var[:], sqs[:], 1.0 / HID)
                tmp = scratch.tile([128, TT], F32, name="ln_tmp", tag="ln_tmp")
                nc.vector.tensor_tensor(tmp[:], mu[:], mu[:], op=ALU.mult)
                nc.vector.tensor_tensor(var[:], var[:], tmp[:], op=ALU.subtract)
                nc.vector.tensor_scalar_add(var[:], var[:], EPS)
                nc.scalar.activation(var[:], var[:], AF.Sqrt)
                nc.vector.reciprocal(rstd[:], var[:])
                for t in range(TT):
                    nc.vector.tensor_scalar(dst_sb[:, t, :], src_sb[:, t, :],
                                            mu[:, t:t + 1], None, op0=ALU.subtract)
                    nc.vector.tensor_scalar(dst_sb[:, t, :], dst_sb[:, t, :],
                                            rstd[:, t:t + 1], None, op0=ALU.mult)

            def transpose_at(src_sb, dst_sb, scratch, psname):
                """src [128, TT, HID] A-layout -> dst [128, KT, NPAD] T-layout."""
                with tc.tile_pool(name=psname, bufs=2, space="PSUM") as pst:
                    for t in range(TT):
                        for k in range(KT):
                            pt = pst.tile([128, 128], F32, name="tr_ps", tag="tr")
                            nc.tensor.transpose(pt[:], src_sb[:, t, k * 128:(k + 1) * 128], ident[:])
                            nc.scalar.copy(dst_sb[:, k, t * 128:(t + 1) * 128], pt[:])

            def router(xT_sb, rw_d, rb_np_name, gates_sb, scratch, psname):
                """xT [128, KT, NPAD]; write gates [128, TT, E]."""
                rw_sb = scratch.tile([128, KT, E], F32, name=psname + "_rw", tag="rw")
                nc.sync.dma_start(out=rw_sb[:], in_=rw_d[:].rearrange("(k p) e -> p k e", p=128))
                with tc.tile_pool(name=psname, bufs=2, space="PSUM") as psr:
                    for t in range(TT):
                        lp = psr.tile([128, E], F32, name="lg_ps", tag="lg")
                        for k in range(KT):
                            nc.tensor.matmul(lp[:], xT_sb[:, k, t * 128:(t + 1) * 128],
                                             rw_sb[:, k, :], start=(k == 0), stop=(k == KT - 1))
                        lg = scratch.tile([128, E], F32, name="lg_sb", tag="lg_sb", bufs=2)
                        nc.vector.tensor_copy(lg[:], lp[:])
                        tv = scratch.tile([128, 8], F32, name="tv", tag="tv", bufs=2)
                        ti = scratch.tile([128, 8], U32, name="ti", tag="ti", bufs=2)
                        nc.vector.max_with_indices(tv[:], ti[:], lg[:])
                        tif = scratch.tile([128, 8], F32, name="tif", tag="tif", bufs=2)
                        nc.vector.tensor_copy(tif[:], ti[:])
                        d = scratch.tile([128, 1], F32, name="rt_d", tag="rt_d", bufs=2)
                        nc.vector.tensor_tensor(d[:], tv[:, 1:2], tv[:, 0:1], op=ALU.subtract)
                        ed = scratch.tile([128, 1], F32, name="rt_ed", tag="rt_ed", bufs=2)
                        nc.scalar.activation(ed[:], d[:], AF.Exp)
                        w1 = scratch.tile([128, 1], F32, name="rt_w1", tag="rt_w1", bufs=2)
                        nc.vector.tensor_scalar_add(w1[:], ed[:], 1.0)
                        nc.vector.reciprocal(w1[:], w1[:])
                        w2 = scratch.tile([128, 1], F32, name="rt_w2", tag="rt_w2", bufs=2)
                        nc.vector.tensor_tensor(w2[:], ed[:], w1[:], op=ALU.mult)
                        m1 = scratch.tile([128, E], F32, name="rt_m1", tag="rt_m1", bufs=2)
                        m2 = scratch.tile([128, E], F32, name="rt_m2", tag="rt_m2", bufs=2)
                        nc.vector.tensor_scalar(m1[:], iota8[:], tif[:, 0:1], None, op0=ALU.is_equal)
                        nc.vector.tensor_scalar(m2[:], iota8[:], tif[:, 1:2], None, op0=ALU.is_equal)
                        nc.vector.tensor_scalar(m1[:], m1[:], w1[:, 0:1], None, op0=ALU.mult)
                        nc.vector.scalar_tensor_tensor(gates_sb[:, t, :], m2[:], w2[:, 0:1], m1[:],
                                                       op0=ALU.mult, op1=ALU.add)

            def moe1024(xT_sb, w_d, gates_sb, acc_sb, scratch, psname, overwrite):
                """dense 1024x1024 moe: acc (+)= sum_e gate_e * (x @ W_e).
                acc [128, TT, HID] A-layout."""
                with tc.tile_pool(name=psname, bufs=2, space="PSUM") as psm:
                    for e in range(E):
                        we = scratch.tile([128, KT, HID], F32, name="moe_we", tag="moe_we", bufs=1)
                        nc.sync.dma_start(out=we[:], in_=w_d[e].rearrange("(k p) h -> p k h", p=128))
                        for t in range(TT):
                            yp = psm.tile([128, HID], F32, name="moe_y", tag="moe_y")
                            for k in range(KT):
                                for n2 in range(2):
                                    nc.tensor.matmul(
                                        yp[:, n2 * 512:(n2 + 1) * 512],
                                        xT_sb[:, k, t * 128:(t + 1) * 128],
                                        we[:, k, n2 * 512:(n2 + 1) * 512],
                                        start=(k == 0), stop=(k == KT - 1))
                            g = gates_sb[:, t, e:e + 1]
                            if e == 0 and overwrite:
                                nc.vector.tensor_scalar(acc_sb[:, t, :], yp[:], g, None, op0=ALU.mult)
                            else:
                                nc.vector.scalar_tensor_tensor(acc_sb[:, t, :], yp[:], g,
                                                               acc_sb[:, t, :],
                                                               op0=ALU.mult, op1=ALU.add)

            # ============ ln1 + lnxT ============
            lnxT = xT_tile("lnxT")
            with tc.tile_pool(name="ph1", bufs=1) as ph1:
                lnx = ph1.tile([128, TT, HID], F32, name="lnx")
                layernorm(xs_sb, lnx, ph1)
                transpose_at(lnx, lnxT, ph1, "pstr1")

            gates_v = main.tile([128, TT, E], F32, name="gates_v")
            with tc.tile_pool(name="ph2", bufs=1) as ph2:
                router(lnxT, vrw, "vrb", gates_v, ph2, "psrv")

            # ============ v-moe (dense) ============
            v_acc = tok_tile("v_acc")
            with tc.tile_pool(name="ph3", bufs=1) as ph3:
                moe1024(lnxT, vw, gates_v, v_acc, ph3, "psmv", overwrite=True)

            # ============ q/k projection + rope, then attention ============
            with tc.tile_pool(name="atp", bufs=1) as atp:
              qT = atp.tile([128, KT, NPAD], F32, name="qT")
              kT = atp.tile([128, KT, NPAD], F32, name="kT")
              vaug = atp.tile([128, TT, NH * 65], F32, name="vaug")
              with tc.tile_pool(name="ph4", bufs=1) as ph4:
                for (wd, bT, dst) in ((qw, qbT, qT), (kw, kbT, kT)):
                    bsb = ph4.tile([128, KT], F32, name="qk_b", tag="qk_b", bufs=2)
                    nc.sync.dma_start(out=bsb[:], in_=bT[:])
                    with tc.tile_pool(name="psqk", bufs=2, space="PSUM") as psq:
                        for dt in range(KT):
                            qp = psq.tile([128, NPAD], F32, name="qk_ps", tag="qk_ps")
                            for k in range(KT):
                                wsb = ph4.tile([128, 128], F32, name="qk_w", tag="qk_w", bufs=3)
                                nc.sync.dma_start(
                                    out=wsb[:],
                                    in_=wd[k * 128:(k + 1) * 128, dt * 128:(dt + 1) * 128])
                                for nn, (n0, nw) in enumerate(((0, 512), (512, 384))):
                                    nc.tensor.matmul(qp[:, n0:n0 + nw],
                                                     wsb[:],
                                                     lnxT[:, k, n0:n0 + nw],
                                                     start=(k == 0), stop=(k == KT - 1))
                            raw = ph4.tile([128, NPAD], F32, name="qk_raw", tag="qk_raw", bufs=2)
                            nc.scalar.activation(raw[:], qp[:], AF.Identity, bias=bsb[:, dt:dt + 1])
                            rot = ph4.tile([128, NPAD], F32, name="qk_rot", tag="qk_rot", bufs=2)
                            for hh in range(2):
                                b0 = hh * 64
                                nc.sync.dma_start(out=rot[b0:b0 + 32, :], in_=raw[b0 + 32:b0 + 64, :])
                                nc.sync.dma_start(out=rot[b0 + 32:b0 + 64, :], in_=raw[b0:b0 + 32, :])
                            t1 = ph4.tile([128, NPAD], F32, name="qk_t1", tag="qk_t1", bufs=2)
                            nc.vector.tensor_tensor(t1[:], raw[:], ropeC[:], op=ALU.mult)
                            nc.vector.tensor_tensor(rot[:], rot[:], ropeS[:], op=ALU.mult)
                            nc.vector.tensor_tensor(dst[:, dt, :], t1[:], rot[:], op=ALU.add)

              # ============ attention ============
              o_sb = tok_tile("o_sb")
              with tc.tile_pool(name="ph5", bufs=1) as ph5:
                for t in range(TT):
                    nc.vector.memset(vaug[:, t, :], 1.0)
                    for h in range(NH):
                        nc.vector.tensor_copy(vaug[:, t, h * 65:h * 65 + 64],
                                              v_acc[:, t, h * 64:(h + 1) * 64])
                with tc.tile_pool(name="psat", bufs=2, space="PSUM") as psa, \
                     tc.tile_pool(name="psao", bufs=2, space="PSUM") as pso:
                    for h in range(NH):
                        hb = (h % 2) * 64
                        hq = h // 2
                        for qt in range(TT):
                            op_ = pso.tile([128, 65], F32, name="o_ps", tag="o_ps")
                            for kt in range(qt + 1):
                                sp = psa.tile([128, 128], F32, name="s_ps", tag="s_ps")
                                nc.tensor.matmul(sp[:],
                                                 kT[hb:hb + 64, hq, kt * 128:(kt + 1) * 128],
                                                 qT[hb:hb + 64, hq, qt * 128:(qt + 1) * 128],
                                                 start=True, stop=True)
                                if kt == qt:
                                    nc.vector.tensor_tensor(sp[:], sp[:], diag[:], op=ALU.add)
                                es = ph5.tile([128, 128], F32, name="es", tag="es", bufs=3)
                                nc.scalar.activation(es[:], sp[:], AF.Exp, scale=float(HD) ** -0.5)
                                nc.tensor.matmul(op_[:], es[:], vaug[:, kt, h * 65:(h + 1) * 65],
                                                 start=(kt == 0), stop=(kt == qt))
                            rc = ph5.tile([128, 1], F32, name="rc", tag="rc", bufs=2)
                            nc.vector.reciprocal(rc[:], op_[:, 64:65])
                            nc.vector.tensor_scalar(o_sb[:, qt, h * 64:(h + 1) * 64],
                                                    op_[:, 0:64], rc[:, 0:1], None, op0=ALU.mult)

            # ============ pmoe ============
            oT = xT_tile("oT")
            with tc.tile_pool(name="ph6", bufs=1) as ph6:
                transpose_at(o_sb, oT, ph6, "pstr2")
                gates_p = ph6.tile([128, TT, E], F32, name="gates_p")
                router(oT, prw, "prb", gates_p, ph6, "psrp")
                # p accumulates straight into xs_sb (residual add)
                moe1024(oT, pw, gates_p, xs_sb, ph6, "psmp", overwrite=False)

            # ============ ln2 + ff ============
            lnx2T = xT_tile("lnx2T")
            with tc.tile_pool(name="ph7", bufs=1) as ph7:
                lnx2 = ph7.tile([128, TT, HID], F32, name="lnx2")
                layernorm(xs_sb, lnx2, ph7)
                transpose_at(lnx2, lnx2T, ph7, "pstr3")
            gates_f = main.tile([128, TT, E], F32, name="gates_f")
            with tc.tile_pool(name="ph8", bufs=1) as ph8:
                router(lnx2T, frw, "frb", gates_f, ph8, "psrf")

            with tc.tile_pool(name="ph9", bufs=1) as ph9:
                for e in range(E):
                    for (c0, cw) in NCH:
                        ctt = cw // 128
                        h_sb = ph9.tile([128, FD, 512], F32, name="ff_h", tag="ff_h")
                        with tc.tile_pool(name="psf1", bufs=2, space="PSUM") as pf1:
                            for d in range(FD):
                                w1t = ph9.tile([128, KT, 128], F32, name="ff_w1", tag="ff_w1", bufs=3)
                                nc.sync.dma_start(
                                    out=w1t[:], in_=fw1r[e, d].rearrange("k p q -> p k q"))
                                hp = pf1.tile([128, 512], F32, name="ff_hps", tag="ff_hps")
                                for k in range(KT):
                                    nc.tensor.matmul(hp[:, :cw], w1t[:, k, :],
                                                     lnx2T[:, k, c0:c0 + cw],
                                                     start=(k == 0), stop=(k == KT - 1))
                                erf = ph9.tile([128, 512], F32, name="ff_erf", tag="ff_erf", bufs=2)
                                nc.scalar.activation(erf[:, :cw], hp[:, :cw], AF.Erf,
                                                     scale=0.7071067811865476)
                                # h = (erf + 1) * pre  (the 0.5 lives in fw2)
                                nc.vector.scalar_tensor_tensor(h_sb[:, d, :cw], erf[:, :cw], 1.0,
                                                               hp[:, :cw], op0=ALU.add, op1=ALU.mult)
                        with tc.tile_pool(name="psf2", bufs=1, space="PSUM") as pf2:
                            yps = [pf2.tile([128, HID], F32, name=f"ff_y{i}", tag=f"ff_y{i}")
                                   for i in range(ctt)]
                            for d in range(FD):
                                w2t = ph9.tile([128, HID], F32, name="ff_w2", tag="ff_w2", bufs=3)
                                nc.sync.dma_start(out=w2t[:], in_=fw2[e, d * 128:(d + 1) * 128, :])
                                for i in range(ctt):
                                    for n2 in range(2):
                                        nc.tensor.matmul(
                                            yps[i][:, n2 * 512:(n2 + 1) * 512],
                                            h_sb[:, d, i * 128:(i + 1) * 128],
                                            w2t[:, n2 * 512:(n2 + 1) * 512],
                                            start=(d == 0), stop=(d == FD - 1))
                            for i in range(ctt):
                                gt = c0 // 128 + i
                                nc.vector.scalar_tensor_tensor(
                                    xs_sb[:, gt, :], yps[i][:], gates_f[:, gt, e:e + 1],
                                    xs_sb[:, gt, :], op0=ALU.mult, op1=ALU.add)

            # ============ head + writeback ============
            xsT = xT_tile("xsT")
            with tc.tile_pool(name="ph10", bufs=1) as ph10:
                transpose_at(xs_sb, xsT, ph10, "pstr4")
                ow_sb = ph10.tile([128, KT, VOCAB], F32, name="ow_sb")
                nc.sync.dma_start(out=ow_sb[:], in_=ow[:].rearrange("(k p) v -> p k v", p=128))
                with tc.tile_pool(name="psh", bufs=2, space="PSUM") as psh:
                    for t in range(TT):
                        lp = psh.tile([128, VOCAB], F32, name="hd_ps", tag="hd_ps")
                        for k in range(KT):
                            nc.tensor.matmul(lp[:], xsT[:, k, t * 128:(t + 1) * 128],
                                             ow_sb[:, k, :], start=(k == 0), stop=(k == KT - 1))
                        lg = ph10.tile([128, VOCAB], F32, name="hd_sb", tag="hd_sb", bufs=2)
                        nc.vector.tensor_copy(lg[:], lp[:])
                        nc.sync.dma_start(out=lg_out[t * 128:(t + 1) * 128, :], in_=lg[:])
                        nc.sync.dma_start(out=xs_out[t * 128:(t + 1) * 128, :], in_=xs_sb[:, t, :])

    nc.finalize()
    return nc


# ---------------------------------------------------------------- runner

def _make_runner(nc, n_cores):
    import jax
    import numpy as np
    from concourse import bass2jax, mybir
    from jax.sharding import Mesh, PartitionSpec, NamedSharding
    from jax.experimental.shard_map import shard_map

    bass2jax.install_neuronx_cc_hook()

    partition_name = nc.partition_id_tensor.name if nc.partition_id_tensor else None
    in_names, out_names, out_avals, zero_outs = [], [], [], []
    for alloc in nc.m.functions[0].allocations:
        if not isinstance(alloc, mybir.MemoryLocationSet):
            continue
        name = alloc.memorylocations[0].name
        if alloc.kind == "ExternalInput":
            if name != partition_name:
                in_names.append(name)
        elif alloc.kind == "ExternalOutput":
            out_names.append(name)
            shape = tuple(alloc.tensor_shape)
            dtype = mybir.dt.np(alloc.dtype)
            out_avals.append(jax.core.ShapedArray(shape, dtype))
            zero_outs.append(np.zeros(shape, dtype))
    n_params = len(in_names)
    all_names = in_names + out_names
    if partition_name is not None:
        all_names = all_names + [partition_name]

    def _body(*args):
        operands = list(args)
        if partition_name is not None:
            operands.append(bass2jax.partition_id_tensor())
        outs = bass2jax._bass_exec_p.bind(
            *operands,
            out_avals=tuple(out_avals),
            in_names=tuple(all_names),
            out_names=tuple(out_names),
            lowering_input_output_aliases=(),
            sim_require_finite=True,
            sim_require_nnan=True,
            nc=nc,
        )
        return tuple(outs)

    devices = jax.devices()[:n_cores]
    mesh = Mesh(np.asarray(devices), ("core",))
    spec = PartitionSpec("core")
    n_outs = len(out_names)
    donate = tuple(range(n_params, n_params + n_outs))
    sharded = jax.jit(
        shard_map(_body, mesh=mesh, in_specs=(spec,) * (n_params + n_outs),
                  out_specs=(spec,) * n_outs, check_rep=False),
        donate_argnums=donate, keep_unused=True)
    sharding = NamedSharding(mesh, spec)

    def put(arr_percore):
        """arr_percore: np [n_cores*dim0, ...] -> committed sharded jax array"""
        return jax.device_put(arr_percore, sharding)

    def run(input_dict_arrays):
        args = [input_dict_arrays[n] for n in in_names]
        args += [np.concatenate([z] * n_cores, axis=0) for z in zero_outs]
        outs = sharded(*args)
        return {n: np.asarray(o) for n, o in zip(out_names, outs)}

    return put, run


# ---------------------------------------------------------------- entry

def kernel(x, condition, params):
    import jax
    if "built" not in _cache:
        nc = _build_block()
        put, run = _make_runner(nc, B)
        _cache["built"] = (put, run)
    put, run = _cache["built"]

    ident, diag, iota8, ropeC, ropeS = _consts()
    wpar = _prep_weights(params)
    xs = _embed(x, condition, params)            # [B, NPAD, HID]

    def rep(a):
        return np.concatenate([np.asarray(a, np.float32)] * B, axis=0)

    consts = {"identc": rep(ident), "diagc": rep(diag), "iota8c": rep(iota8),
              "ropeCc": rep(ropeC), "ropeSc": rep(ropeS)}
    consts = {k: put(v) for k, v in consts.items()}
    wnames = ["qw", "kw", "qbT", "kbT", "vrw", "prw", "frw", "vw", "pw",
              "fw1r", "fw2", "vrb", "prb", "frb"]
    wdev = []
    for par in range(2):
        w = wpar[par]
        d = {n: put(rep(w[n])) for n in wnames}
        wdev.append(d)
    ow_dev = put(rep(_np(params["out"][0])))
    ob = _np(params["out"][1])

    cur = np.concatenate([xs[i] for i in range(B)], axis=0)   # [B*NPAD, HID]
    lg = None
    for blk in range(NB):
        ins = dict(consts)
        ins.update(wdev[blk % 2])
        ins["ow"] = ow_dev
        ins["xs_in"] = cur
        out = run(ins)
        cur = out["xs_out"]
        lg = out["lg_out"]

    lg = lg.reshape(B, NPAD, VOCAB)[:, :N, :]
    if not np.all(ob == 0):
        lg = lg + ob
    return np.ascontiguousarray(lg.astype(np.float32))
